# revision 51
# baseline (speedup 1.0000x reference)
"""Trainium2 Bass kernel for nn_C_Cross_Attention3D (cosine cross-attention,
single query token, 3D conv projections).

Math summary (matches reference exactly):
  x: (2, 768, 32, 32, 32), y: (2, 768, 1, 1, 1)
  kv = kv_w @ x (1x1x1 conv, 1536 out channels), then a *channel-scrambled*
  torch-style reshape turns the flat (1536*32768) conv output per batch into
  32768 rows of 1536 = [k(12 heads x 64) | v(12 heads x 64)].
  Because 2C*N is flattened c-major, row n' = 1536 consecutive flat elements
  = 1536 consecutive spatial positions of ONE output channel (rows start at
  s = 1536*n' mod 32768 within channel c2 = (1536*n')//32768).
  Cosine attention: logits = qhat . khat in [-1,1] -> exp needs no max trick.
  out = sum_n' exp(logit) * v / sum exp(logit), then proj.

Numerics strategy (v2, fp8):
  The softmax is near-uniform (logit std ~ 0.125), so out is ~ an average of
  v over 32768 rows; v's relative quantization error passes through at full
  strength. Decompose exp(l) = 1 + delta (|delta| <~ 1.7): the bulk term
  sum(v) is EXACT and linear in x -- computed host-side in f64 via a periodic
  T-sum formula; the device only computes sum(delta * v), where fp8's ~4%
  error is suppressed by |delta| ~ 0.13. This lets the whole conv run as fp8
  DoubleRow matmuls (2x tensor throughput) while total rel err stays < 1e-2.

Device kernel per core: stream 18 fp8 x-strips of 512 positions; per chunk
(16 iters x 4 channel-blocks): 9 fp8 DoubleRow matmuls (3 slices x 3
cin-pairs of 128) into a PSUM (128,1536) row tile; ACT copies k to bf16 and
v to fp8 (paired tiles for DoubleRow); DVE computes q.k and ||k||^2 via bf16
mul + grouped reduce. Every 8 iters a batched sqrt/exp chain produces
delta = exp(logit)-1 in fp8, then DoubleRow matmuls accumulate
O = sum(delta*v) and Z = sum(delta) into a persistent PSUM tile.
Host combines quarters, adds the exact S_v bulk term, and projects.

Sharding: 8 cores = 2 batches x 4 position-quarters (same row/halo/slot
machinery as v1: rows whose 512-aligned start lies in the quarter; q=3 wraps
to position 0 with channel+1 via extra weight slots).
"""

import sys

sys.path.insert(0, "/opt/trn_rl_repo")

import numpy as np
import ml_dtypes

F8 = ml_dtypes.float8_e4m3
BF16 = ml_dtypes.bfloat16

NUM_HEADS = 12
C = 768
N = 32768
TWO_C = 2 * C
EPS = 1e-12
WSCALE = 64.0   # kv_w pre-scale so fp8 products stay well inside e4m3 range
NQ = 4          # position quarters
QLEN = 8192     # positions per quarter
HALO = 1024
XLEN = QLEN + HALO  # 9216
NCHUNK_I = 16   # chunk iters per core (512-aligned starts)
NBLK = 4        # channel blocks of 128 per residue class
NCIN = 6        # input-channel blocks of 128
# softmax/OZ batch boundaries over the 16 chunk iters; smaller final batches
# shrink the end-of-kernel stats-chain stall
BATCHES = [(0, 4), (4, 8), (8, 12), (12, 14), (14, 16)]

_CACHED = {}
_LAST_IN_MAPS = None


def _class_of_n(n):
    # chunk start s = 512*n; s%1536 = 512*(n%3)
    # 0 -> channels c2%3==0 ; 512 -> c2%3==2 ; 1024 -> c2%3==1
    return {0: 0, 1: 2, 2: 1}[n % 3]


def _slot_classes(q):
    cls = [_class_of_n(16 * q + sigma) for sigma in range(3)]
    x1 = cls[0] if q < 3 else _class_of_n(16 * 3 + 15) + 1
    x2 = cls[2] if q < 3 else _class_of_n(16 * 3 + 14) + 1
    return cls + [x1, x2]


def _slot_for(i, t):
    if (i, t) in ((15, 1), (15, 2)):
        return 3
    if (i, t) == (14, 2):
        return 4
    return i % 3


def _build_program(debug_dump=False):
    import concourse.tile as tile
    from concourse import bacc, mybir

    f32 = mybir.dt.float32
    f8 = mybir.dt.float8e4
    bf = mybir.dt.bfloat16

    nc = bacc.Bacc("TRN2", target_bir_lowering=False, debug=False, num_devices=8)

    # x slice, fp8, viewed (cin_blk*128, XLEN)
    xs = nc.dram_tensor("xs", [C, XLEN], f8, kind="ExternalInput")
    # weight slots, fp8, partition-major: [slot, g, a(cin in blk), k(blk), b(c2)]
    wts = nc.dram_tensor("wts", [5, NBLK, 128, NCIN, 128], f8, kind="ExternalInput")
    # qhat row (1, 768) bf16
    qh = nc.dram_tensor("qh", [1, C], bf, kind="ExternalInput")
    # ones pair for the Z column matmuls (bf16)
    on = nc.dram_tensor("on", [1, 2], bf, kind="ExternalInput")
    out = nc.dram_tensor("out", [NUM_HEADS, 1024], f32, kind="ExternalOutput")
    # delta slabs, dumped per half; the host derives Z = N + sum(delta) from
    # these (a device-side Z via 1-col matmuls interleaved with the O matmuls
    # numerically corrupts the O accumulation -- hardware hazard, see notes)
    dwout = nc.dram_tensor("dwout", [NCHUNK_I, 128, NBLK * NUM_HEADS], bf,
                           kind="ExternalOutput")
    dbg = None
    if debug_dump:
        dbg = {
            "dbgv": nc.dram_tensor("dbgv", [NCHUNK_I, 2, 128, 2 * C], bf,
                                   kind="ExternalOutput"),
        }

    xs_r = xs.ap().rearrange("(k p) n -> p k n", p=128)  # (128, 6, 9216)

    with tile.TileContext(nc) as tc:
        _emit_body(tc, nc, mybir, xs_r, wts, qh, on, out, dwout, dbg)

    nc.compile()
    return nc


def _emit_body(tc, nc, mybir, xs_r, wts, qh, on, out, dwout, dbg=None):
    import concourse.bass as bass

    f32 = mybir.dt.float32
    f8 = mybir.dt.float8e4
    bf = mybir.dt.bfloat16
    AF = mybir.ActivationFunctionType
    ALU = mybir.AluOpType
    DR = mybir.MatmulPerfMode.DoubleRow

    singles = tc.alloc_tile_pool(name="singles", bufs=1)
    xpool = tc.alloc_tile_pool(name="xpool", bufs=5)
    wpool = tc.alloc_tile_pool(name="wpool", bufs=1)
    kpool = tc.alloc_tile_pool(name="kpool", bufs=3)
    vpool = tc.alloc_tile_pool(name="vpool", bufs=18)
    spool = tc.alloc_tile_pool(name="spool", bufs=2)
    pspool = tc.alloc_tile_pool(name="pspool", bufs=2, space="PSUM")
    ozpool = tc.alloc_tile_pool(name="ozpool", bufs=1, space="PSUM")

    # ---- warmup scratch (no DMA deps): keeps the PE HAM window busy so the
    # real matmuls start at 2.4 GHz instead of 1.2 ----
    wmw = singles.tile([128, 2, 128], f8)
    wmx = singles.tile([128, 2, 512], f8)
    nc.gpsimd.memset(wmw[:], 0)
    nc.gpsimd.memset(wmx[:], 0)

    # ---- DMA issue order is startup-latency-critical: strip0 + slot-0
    # weights first (chunk 0 needs only those), then the rest interleaved ----
    x_strips = [None] * (NCHUNK_I + 2)

    def load_strip(s):
        t = xpool.tile([128, NCIN, 512], f8, tag="xstrip")
        nc.sync.dma_start(t[:], xs_r[:, :, 512 * s:512 * (s + 1)])
        x_strips[s] = t

    w_sb = {}

    def load_w(sigma, g):
        t = wpool.tile([128, NCIN, 128], f8, tag=f"w{sigma}_{g}")
        nc.sync.dma_start(t[:], wts.ap()[sigma, g])
        w_sb[(sigma, g)] = t

    # qhat replicated x4 so one TT can multiply a whole chunk-iter's k quad
    qhat4 = singles.tile([128, NBLK, C], bf)
    qa = qh.ap()
    ones = singles.tile([128, 2], bf)
    oa = on.ap()

    load_strip(0)
    for g in range(NBLK):
        load_w(0, g)
    load_strip(1)
    load_strip(2)
    load_strip(3)
    for g in range(NBLK):
        nc.sync.dma_start(qhat4[:, g, :], bass.AP(tensor=qa.tensor, offset=qa.offset, ap=[[0, 128], [1, C]]))
    nc.sync.dma_start(ones[:], bass.AP(tensor=oa.tensor, offset=oa.offset, ap=[[0, 128], [1, 2]]))
    for g in range(NBLK):
        load_w(1, g)
    load_strip(4)
    for g in range(NBLK):
        load_w(2, g)
    load_strip(5)
    load_strip(6)
    for sigma in range(3, 5):
        for g in range(NBLK):
            load_w(sigma, g)
    for s in range(7, NCHUNK_I + 2):
        load_strip(s)

    # persistent O/Z accumulator: cols [0,768) = O_delta, col 768 = Z_delta
    oz = ozpool.tile([NUM_HEADS, 1024], f32)

    # per-(i,g,h) stat slabs
    kd_slab = singles.tile([128, NCHUNK_I, NBLK, NUM_HEADS], f32)
    nm_slab = singles.tile([128, NCHUNK_I, NBLK, NUM_HEADS], f32)

    # ---- PE warmup: ~10 dummy DoubleRow matmuls on scratch data ----
    ps_warm = pspool.tile([128, 3 * 512], f32, tag="rows")
    for _ in range(6):
        nc.tensor.matmul(ps_warm[:, 0:512], wmw[:], wmx[:],
                         start=True, stop=True, perf_mode=DR)

    ozs = [False]  # oz accumulation started?

    def oz_batch(b0, b1, dwb, last):
        # bf16 O matmuls for chunk iters [b0, b1)
        for ii in range(b1 - b0):
            i = b0 + ii
            for g in range(NBLK):
                idx = (ii * NBLK + g) * NUM_HEADS
                lhs = dwb[:, idx:idx + NUM_HEADS]
                vp = vpairs[(i, g // 2)]
                st = not ozs[0]
                sp = (last and ii == b1 - b0 - 1 and g == NBLK - 1)
                nc.tensor.matmul(oz[:, 0:512], lhs, vp[:, g % 2, 0:512],
                                 start=st, stop=sp)
                nc.tensor.matmul(oz[:, 512:768], lhs, vp[:, g % 2, 512:768],
                                 start=st, stop=sp)
                ozs[0] = True

    vpairs = {}
    pending = []  # deferred OZ batch jobs
    batch_end = {b1 - 1: (bi, b0, b1) for bi, (b0, b1) in enumerate(BATCHES)}

    for i in range(NCHUNK_I):
        kbq = kpool.tile([128, NBLK, C], bf, tag="kb")
        for g in range(NBLK):
            # --- conv row tile: 9 fp8 DoubleRow matmuls ---
            # jj outer so the same stationary tile is reused back-to-back
            # when all three slices share a slot (non-crossing chunks)
            ps = pspool.tile([128, 3 * 512], f32, tag="rows")
            slots = [_slot_for(i, t) for t in range(3)]
            if slots[0] == slots[1] == slots[2]:
                wt = w_sb[(slots[0], g)]
                for jj in range(3):
                    for t in range(3):
                        nc.tensor.matmul(
                            ps[:, 512 * t:512 * (t + 1)],
                            wt[:, 2 * jj:2 * jj + 2, :],
                            x_strips[i + t][:, 2 * jj:2 * jj + 2, :],
                            start=(jj == 0),
                            stop=(jj == 2),
                            perf_mode=DR,
                        )
            else:
                for t in range(3):
                    wt = w_sb[(slots[t], g)]
                    for jj in range(3):
                        nc.tensor.matmul(
                            ps[:, 512 * t:512 * (t + 1)],
                            wt[:, 2 * jj:2 * jj + 2, :],
                            x_strips[i + t][:, 2 * jj:2 * jj + 2, :],
                            start=(jj == 0),
                            stop=(jj == 2),
                            perf_mode=DR,
                        )
            # --- k to bf16 quad, v to bf16 (paired) ---
            nc.scalar.copy(kbq[:, g, :], ps[:, 0:C])
            p = g // 2
            if g % 2 == 0:
                vp = vpool.tile([128, 2, C], bf, tag="vp")
                vpairs[(i, p)] = vp
            else:
                vp = vpairs[(i, p)]
            nc.scalar.copy(vp[:, g % 2, :], ps[:, C:TWO_C])
            if dbg is not None:
                nc.sync.dma_start(
                    dbg["dbgv"].ap()[i, p].rearrange("q (s c) -> q s c", s=2)[:, g % 2, :],
                    vp[:, g % 2, :])
        # emit the previous batch's deferred OZ matmuls after this chunk's
        # conv matmuls are queued
        if pending:
            oz_batch(*pending.pop(0))
        # --- bf16 stats for the whole chunk-iter: TT mul, two 2x-packed
        # strided folds (64->32->16), then a 1x reduce over 16 ---
        def stat(src_quad, out_slab):
            f1 = kpool.tile([128, NBLK, NUM_HEADS, 32], bf, tag="f1")
            sv = src_quad[:].rearrange("p g (h d) -> p g h d", d=64)
            nc.vector.tensor_add(f1[:], sv[:, :, :, 0:32], sv[:, :, :, 32:64])
            f2 = kpool.tile([128, NBLK, NUM_HEADS, 16], bf, tag="f2")
            nc.vector.tensor_add(f2[:], f1[:, :, :, 0:16], f1[:, :, :, 16:32])
            nc.vector.tensor_reduce(
                out_slab[:, i, :, :].rearrange("p g h -> p (g h)"),
                f2[:].rearrange("p g h d -> p (g h) d"),
                axis=mybir.AxisListType.X,
                op=ALU.add,
            )

        kq = kpool.tile([128, NBLK, C], bf, tag="kq")
        nc.vector.tensor_mul(kq[:], kbq[:], qhat4[:])
        stat(kq, kd_slab)
        k2 = kpool.tile([128, NBLK, C], bf, tag="k2")
        nc.vector.tensor_mul(k2[:], kbq[:], kbq[:])
        stat(k2, nm_slab)

        if i in batch_end:
            # --- batched softmax chain over (128, (b1-b0)*4*12) ---
            bi, b0, b1 = batch_end[i]
            kdv = kd_slab[:, b0:b1]
            nmv = nm_slab[:, b0:b1]
            W = (b1 - b0) * NBLK * NUM_HEADS
            WMAX = 4 * NBLK * NUM_HEADS
            nr = spool.tile([128, WMAX], f32, tag="nr")
            nc.scalar.sqrt(nr[:, 0:W], nmv.rearrange("p i g h -> p (i g h)"))
            nc.vector.tensor_scalar_max(nr[:, 0:W], nr[:, 0:W], EPS)
            rc = spool.tile([128, WMAX], f32, tag="rc")
            nc.vector.reciprocal(rc[:, 0:W], nr[:, 0:W])
            lg = spool.tile([128, WMAX], f32, tag="lg")
            nc.vector.tensor_mul(
                lg[:, 0:W], kdv.rearrange("p i g h -> p (i g h)"), rc[:, 0:W])
            we = spool.tile([128, WMAX], f32, tag="we")
            nc.scalar.activation(we[:, 0:W], lg[:, 0:W], AF.Exp)
            dwb = spool.tile([128, WMAX], bf, tag="dwb")
            nc.vector.tensor_scalar_add(dwb[:, 0:W], we[:, 0:W], -1.0)
            nc.sync.dma_start(
                dwout.ap()[b0:b1].rearrange("i p w -> p i w"),
                dwb[:, 0:W].rearrange("p (i w) -> p i w", i=b1 - b0))
            # --- O accumulation: deferred one chunk so the next chunk's conv
            # matmuls aren't blocked behind the stats chain; last batch
            # emits immediately (nothing follows it) ---
            job = (b0, b1, dwb, bi == len(BATCHES) - 1)
            if bi == len(BATCHES) - 1:
                oz_batch(*job)
            else:
                pending.append(job)

    oz_sb = singles.tile([NUM_HEADS, 1024], f32)
    nc.vector.tensor_copy(oz_sb[:], oz[:])
    nc.sync.dma_start(out.ap(), oz_sb[:])

    for p in (ozpool, pspool, spool, vpool, kpool, wpool, xpool, singles):
        p.release()


def _gather_weights(kv_w, q):
    """fp8 weight slots, partition-major layout [slot, g, a, k, b]."""
    wts = np.empty((5, NBLK, 128, NCIN, 128), F8)
    wsc = (kv_w * WSCALE).astype(np.float32)
    for sigma, r in enumerate(_slot_classes(q)):
        chans = np.arange(512) * 3 + r
        blk_w = wsc[chans, :]  # (512, 768)
        for g in range(NBLK):
            sub = blk_w[128 * g:128 * (g + 1), :]  # (b, cin_full)
            # [a(cin%128), k(cin//128), b]
            wts[sigma, g] = sub.reshape(128, NCIN, 128).transpose(2, 1, 0).astype(F8)
    return np.ascontiguousarray(wts)


def _host_sv(xf_b, kv_w):
    """Exact sum_n' v[n', :] for one batch, in f64.

    S_v[j] = sum_{c} kv_w[c,:] . T[:, (768 + j + 1024*(c%3)) % 1536]
    where T[cin, t] = sum_m x[cin, t + 1536*m]  (zero-padded past N).
    """
    xpad = np.zeros((C, 22 * 1536), np.float64)
    xpad[:, :N] = xf_b
    T = xpad.reshape(C, 22, 1536).sum(1)           # (768, 1536)
    wr = np.stack([kv_w[r::3].astype(np.float64).sum(0) for r in range(3)])
    j = np.arange(C)
    Sv = np.zeros(C)
    for r in range(3):
        cols = (768 + j + 1024 * r) % 1536
        Sv += wr[r] @ T[:, cols]
    return Sv


def _host_reference(x, y, q_w, q_b, kv_w, kv_b, proj_w, proj_b):
    """Exact numpy fallback (used only for inputs the device path doesn't
    cover, e.g. nonzero kv bias)."""
    B = x.shape[0]
    xf = x.reshape(B, C, N).astype(np.float64)
    outs = []
    for b in range(B):
        qv = q_w.astype(np.float64) @ y[b, :, 0, 0, 0].astype(np.float64) + q_b
        qm = qv.reshape(NUM_HEADS, 64)
        qhat = qm / np.maximum(np.linalg.norm(qm, axis=1, keepdims=True), EPS)
        conv = kv_w.astype(np.float64) @ xf[b] + kv_b.astype(np.float64)[:, None]
        rows = conv.reshape(-1).reshape(N, 2, C)
        k = rows[:, 0].reshape(N, NUM_HEADS, 64)
        v = rows[:, 1]
        kn = np.maximum(np.linalg.norm(k, axis=2), EPS)
        logits = np.einsum('nhd,hd->nh', k, qhat) / kn
        w = np.exp(logits - logits.max(0))
        w /= w.sum(0)
        O = np.einsum('nh,nc->hc', w, v)
        attn = np.empty(C)
        for h in range(NUM_HEADS):
            attn[h * 64:(h + 1) * 64] = O[h, h * 64:(h + 1) * 64]
        outs.append(proj_w.astype(np.float64) @ attn + proj_b)
    return np.stack(outs).astype(np.float32).reshape(B, C, 1, 1, 1)


def kernel(x, y, q_w, q_b, kv_w, kv_b, proj_w, proj_b):
    from concourse.bass_utils import run_bass_kernel_spmd

    x = np.asarray(x, dtype=np.float32)
    y = np.asarray(y, dtype=np.float32)
    q_w = np.asarray(q_w, dtype=np.float32)
    q_b = np.asarray(q_b, dtype=np.float32)
    kv_w = np.asarray(kv_w, dtype=np.float32)
    kv_b = np.asarray(kv_b, dtype=np.float32)
    proj_w = np.asarray(proj_w, dtype=np.float32)
    proj_b = np.asarray(proj_b, dtype=np.float32)

    B = x.shape[0]
    if np.any(kv_b != 0.0) or B != 2:
        return _host_reference(x, y, q_w, q_b, kv_w, kv_b, proj_w, proj_b)
    xf = x.reshape(B, C, N)

    if "prog" not in _CACHED:
        _CACHED["prog"] = _build_program()
    nc = _CACHED["prog"]

    # host: qhat per batch (bf16 for the device)
    qhats = []
    for b in range(B):
        qv = q_w @ y[b, :, 0, 0, 0] + q_b
        qm = qv.reshape(NUM_HEADS, 64)
        nrm = np.maximum(np.linalg.norm(qm, axis=1, keepdims=True), EPS)
        qhats.append((qm / nrm).reshape(C))

    xq = xf.astype(F8)  # quantize once for both batches

    in_maps = []
    wts_cache = {}
    for core in range(8):
        b, q = divmod(core, NQ)
        lo = QLEN * q
        hi = lo + XLEN
        if hi <= N:
            xs = xq[b][:, lo:hi]
        else:
            xs = np.concatenate([xq[b][:, lo:], xq[b][:, :hi - N]], axis=1)
        if q not in wts_cache:
            wts_cache[q] = _gather_weights(kv_w, q)
        in_maps.append({
            "xs": np.ascontiguousarray(xs),
            "wts": wts_cache[q],
            "qh": qhats[b].astype(BF16).reshape(1, C),
            "on": np.ones((1, 2), BF16),
        })

    global _LAST_IN_MAPS, _LAST_RES
    _LAST_IN_MAPS = in_maps
    res = run_bass_kernel_spmd(nc, in_maps, core_ids=list(range(8)))
    _LAST_RES = res

    outs = []
    for b in range(B):
        O = np.zeros((NUM_HEADS, 768), np.float64)
        Zd = np.zeros((NUM_HEADS,), np.float64)
        for q in range(NQ):
            rr = res.results[NQ * b + q]
            O += rr["out"][:, 0:768]
            dwv = rr["dwout"].astype(np.float64)
            Zd += dwv.reshape(NCHUNK_I, 128, NBLK, NUM_HEADS).sum((0, 1, 2))
        Sv = _host_sv(xf[b], kv_w)
        Z = float(N) + Zd
        attn = np.empty((C,), np.float64)
        for h in range(NUM_HEADS):
            cols = slice(h * 64, (h + 1) * 64)
            attn[cols] = (Sv[cols] + O[h, cols] / WSCALE) / Z[h]
        outs.append(proj_w.astype(np.float64) @ attn + proj_b)
    return np.stack(outs).astype(np.float32).reshape(B, C, 1, 1, 1)
